# revision 27
# baseline (speedup 1.0000x reference)
"""MoELoRALinear Trainium2 kernel (8-core data-parallel, Bass/Tile).

Math (per token t, out feature o):
    out[t,o] = x[t,:] @ base_w[o,:] + base_b[o]
             + sum_e softmax_e(x[t,:] @ router_w[e,:]) * SCALE
               * sum_r (x[t,:] @ A[e,r,:]) * B[e,o,r]

Strategy:
  - 8192 tokens sharded 8 ways (1024 tokens/core); weights replicated.
  - All operands shipped as bf16 (half the DMA bytes of fp32r; same PE
    throughput: 1 col/cycle). Output returned bf16, upcast on host.
    Norm rel-err ~2e-3, well inside the 2e-2 gate.
  - Per 128-col chunk: YT[j, tok] = [A;router].T @ x accumulated over all
    16 K-chunks (wide-N matmuls), interleaved with a 4-token-chunk base
    matmul wave (4+2 PSUM banks).
  - Softmax/gating WITHOUT PE transposes: exp on ACT straight from PSUM
    (no max-sub needed: logits are ~N(0,1)); per-token sums / broadcast
    done with tiny matmuls (ones / expansion-matrix lhsT); division and
    gating on DVE. The gated rank-32 projection lands directly in the
    [33, tok] lhsT layout the close matmuls need.
  - Close matmul (K=33: gated proj + ones row for bias) accumulates into
    the same PSUM group as the 16 base K-chunks.
  - DMA rings split: scalar=weights/tables, gpsimd=x stream, sync=outputs.
  - 4 warmup matmuls on a zero tile ramp the PE DVFS p-state during the
    fixed ~7us NEFF prologue + first x-chunk DMA.
"""

import os

import numpy as np

import concourse.bacc as bacc
import concourse.bass as bass
import concourse.mybir as mybir
from concourse.bass_utils import run_bass_kernel_spmd
from concourse.tile import TileContext

SCALE = 16.0 / 8.0  # alpha / r

N_CORES = 8
TOK = 8192  # 4 * 2048 tokens total
TPC = TOK // N_CORES  # tokens per core = 1024
D = 2048  # in features
O = 2048  # out features
E = 4
R = 8
ER = E * R  # 32
J = ER + E  # 36: rank-proj rows + router rows
DC = D // 128  # 16 contraction chunks
OCW = 512  # out-feature chunk width (one PSUM bank)
OC = O // OCW  # 4
TC = TPC // 128  # 8 token chunks per core
WAVE = 4  # wave-A token chunks (PSUM: WAVE + ytps1 + 2 filler groups + 1 ypsum)
KP = 128  # close-matmul contraction rows (33 used, padded to full array)

F32 = mybir.dt.float32
BF16 = mybir.dt.bfloat16

# Results of the last device run (for test harness inspection).
last_run_info: dict = {}

_cached = None


def _build_program():
    nc = bacc.Bacc()

    xt_d = nc.declare_dram_parameter("xt", [128, DC * TPC], BF16, isOutput=False)
    wt_d = nc.declare_dram_parameter("wt", [OC, 128, DC * OCW], BF16, isOutput=False)
    w1t_d = nc.declare_dram_parameter("w1t", [128, DC * J], BF16, isOutput=False)
    bcat_d = nc.declare_dram_parameter("bcat", [KP, O], BF16, isOutput=False)
    cst_d = nc.declare_dram_parameter("cst", [E, ER + 4], BF16, isOutput=False)
    out_d = nc.declare_dram_parameter("out", [OC, TC, 128, OCW], BF16, isOutput=True)

    MUL = mybir.AluOpType.mult

    with TileContext(nc) as tc:
        with (
            tc.tile_pool(name="cpool", bufs=1) as cpool,
            tc.tile_pool(name="wpool", bufs=4) as wpool,
            tc.tile_pool(name="opool", bufs=6) as opool,
            tc.tile_pool(name="mpsum", bufs=7, space="PSUM") as mpsum,
            tc.tile_pool(name="ypsum", bufs=1, space="PSUM") as ypsum,
        ):
            # x gets a dedicated ring (gpsimd) so its stream isn't halved by
            # the weight traffic; everything else rides the scalar ring in
            # consumption order (tables, w0 pieces, then bcat/w1..w3 which
            # are not needed before ~60us). Outputs go on sync.
            cstr = cpool.tile([E, ER + 4], BF16)
            nc.scalar.dma_start(out=cstr, in_=cst_d[:, :])
            w1tr = cpool.tile([128, DC * J], BF16)
            nc.scalar.dma_start(out=w1tr, in_=w1t_d[:, :])

            wts = {
                oc: wpool.tile([128, DC * OCW], BF16, name=f"wtr{oc}", tag="wtr")
                for oc in range(OC)
            }
            # w0 in 4 k-block pieces (wave A unlocks per piece).
            for k in range(4):
                nc.scalar.dma_start(
                    out=wts[0][:, k * 4 * OCW : (k + 1) * 4 * OCW],
                    in_=wt_d[0, :, k * 4 * OCW : (k + 1) * 4 * OCW],
                )
            bcatr = cpool.tile([KP, O], BF16)
            nc.scalar.dma_start(out=bcatr, in_=bcat_d[:, :])
            for oc in range(1, OC):
                for h in range(2):
                    nc.scalar.dma_start(
                        out=wts[oc][:, h * 8 * OCW : (h + 1) * 8 * OCW],
                        in_=wt_d[oc, :, h * 8 * OCW : (h + 1) * 8 * OCW],
                    )

            xtr = cpool.tile([128, DC * TPC], BF16)

            def load_x(eng, d0, nd):
                eng.dma_start(
                    out=xtr[:, d0 * TPC : (d0 + nd) * TPC],
                    in_=xt_d[:, d0 * TPC : (d0 + nd) * TPC],
                )

            # Early chunks fine-grained, later ones paired, all on the fast
            # 16-engine sync ring.
            for dc in range(4):
                load_x(nc.sync, dc, 1)
            for dp in range(2, 8):
                load_x(nc.sync, 2 * dp, 2)

            # Warmup: ramp the PE p-state on a zero tile while the first
            # x chunk is still in flight.
            warm_sb = cpool.tile([128, OCW], BF16)
            nc.vector.memset(warm_sb, 0.0)
            # Close lhsT padded to 128 contraction rows: 32 gated-proj rows,
            # the ones row (bias), then zeros (avoids 64-row PE tile mode).
            vwtr = cpool.tile([KP, TPC], BF16)
            for p0 in range(ER, KP, 32):
                nc.vector.memset(vwtr[p0 : p0 + 32, :], 0.0)
            nc.vector.memset(vwtr[ER : ER + 1, :], 1.0)
            warmps = ypsum.tile([128, OCW], F32, name="warmps", tag="yb")
            for _ in range(8):
                nc.tensor.matmul(
                    warmps[:, 0:256],
                    lhsT=warm_sb[:, 0:128],
                    rhs=warm_sb[:, 0:256],
                    start=True,
                    stop=True,
                )

            # --- Router/LoRA-down phase: YT[j, tok] accumulated over all
            # dc, interleaved with wave-A base matmuls in DMA arrival order.
            # ytps[1] lives in the mpsum pool so ypsum (bufs=1) can rotate
            # the small gating-chain tiles behind it.
            ytps = [
                ypsum.tile([J, 512], F32, name="ytps0", tag="yb"),
                mpsum.tile([J, 512], F32, name="ytps1", tag="ps"),
            ]
            psA = {
                t: mpsum.tile([128, OCW], F32, name=f"ps0_{t}", tag="ps")
                for t in range(WAVE)
            }
            for k in range(4):
                for dc in range(4 * k, 4 * k + 4):
                    for th in range(2):
                        nc.tensor.matmul(
                            ytps[th],
                            lhsT=w1tr[:, dc * J : (dc + 1) * J],
                            rhs=xtr[:, dc * TPC + th * 512 : dc * TPC + (th + 1) * 512],
                            start=(dc == 0),
                            stop=(dc == DC - 1),
                        )
                for dc in range(4 * k, 4 * k + 4):
                    for t in range(WAVE):
                        nc.tensor.matmul(
                            psA[t],
                            lhsT=xtr[:, dc * TPC + t * 128 : dc * TPC + (t + 1) * 128],
                            rhs=wts[0][:, dc * OCW : (dc + 1) * OCW],
                            start=(dc == 0),
                            stop=False,
                        )

            # --- Gating chain (no PE transposes).
            # u = exp(logits) straight off PSUM (no max-sub: logits ~N(0,1));
            # sums4 = 0.5-ones matmul replicates s/SCALE onto 4 rows;
            # r4 = SCALE/s via fast approx recip; g = u*r4 on DVE;
            # gb = E8.T@g broadcasts per-expert gates to the 32 rank rows;
            # vw = Y * gb lands directly in the [32, tok] bf16 close layout.
            y_sb = cpool.tile([ER, TPC], BF16)
            u_sb = cpool.tile([E, TPC], BF16)
            r_sb = cpool.tile([E, TPC], F32)
            g_sb = cpool.tile([E, TPC], BF16)
            for th in range(2):
                nc.vector.tensor_copy(
                    y_sb[:, th * 512 : (th + 1) * 512], ytps[th][0:ER, :]
                )
                nc.scalar.activation(
                    u_sb[:, th * 512 : (th + 1) * 512],
                    ytps[th][ER:J, :],
                    mybir.ActivationFunctionType.Exp,
                )

            def open_group(wtr, t, name):
                ps = mpsum.tile([128, OCW], F32, name=name, tag="ps")
                for dc in range(DC):
                    nc.tensor.matmul(
                        ps,
                        lhsT=xtr[:, dc * TPC + t * 128 : dc * TPC + (t + 1) * 128],
                        rhs=wtr[:, dc * OCW : (dc + 1) * OCW],
                        start=(dc == 0),
                        stop=False,
                    )
                return ps

            def open_group_part(ps, wtr, t, dcs):
                for dc in dcs:
                    nc.tensor.matmul(
                        ps,
                        lhsT=xtr[:, dc * TPC + t * 128 : dc * TPC + (t + 1) * 128],
                        rhs=wtr[:, dc * OCW : (dc + 1) * OCW],
                        start=(dc == 0),
                        stop=False,
                    )

            # The t4/t5/t6 base groups need no new DMA data, so their 48
            # matmuls are interleaved as PE filler around the chain's small
            # matmuls; the single-buf ypsum rotation serializes the chain
            # (sums0 -> recip0 -> sums1 -> recip1 -> gb0 -> vw0 -> gb1 ->
            # vw1), each hop hidden behind ~1.7us of base matmuls.
            def sums_mm(th):
                s = ypsum.tile([E, 512], F32, name=f"sums{th}", tag="yb")
                nc.tensor.matmul(
                    s,
                    lhsT=cstr[0:E, ER : ER + 4],
                    rhs=u_sb[:, th * 512 : (th + 1) * 512],
                    start=True,
                    stop=True,
                )
                return s

            def recip_g(th, s):
                nc.vector.reciprocal_approx_fast(
                    out=r_sb[:, th * 512 : (th + 1) * 512], in_=s
                )
                nc.vector.tensor_tensor(
                    g_sb[:, th * 512 : (th + 1) * 512],
                    u_sb[:, th * 512 : (th + 1) * 512],
                    r_sb[:, th * 512 : (th + 1) * 512],
                    op=MUL,
                )

            def gb_mm(th):
                gb = ypsum.tile([ER, 512], F32, name=f"gb{th}", tag="yb")
                nc.tensor.matmul(
                    gb,
                    lhsT=cstr[0:E, 0:ER],
                    rhs=g_sb[:, th * 512 : (th + 1) * 512],
                    start=True,
                    stop=True,
                )
                return gb

            def vw_tt(th, gb):
                nc.vector.tensor_tensor(
                    vwtr[0:ER, th * 512 : (th + 1) * 512],
                    y_sb[:, th * 512 : (th + 1) * 512],
                    gb,
                    op=MUL,
                )

            psA[WAVE] = open_group(wts[0], WAVE, f"ps0_{WAVE}")
            s0 = sums_mm(0)
            recip_g(0, s0)
            psA[WAVE + 1] = mpsum.tile(
                [128, OCW], F32, name=f"ps0_{WAVE + 1}", tag="ps"
            )
            open_group_part(psA[WAVE + 1], wts[0], WAVE + 1, range(0, 8))
            s1 = sums_mm(1)
            recip_g(1, s1)
            open_group_part(psA[WAVE + 1], wts[0], WAVE + 1, range(8, DC))
            gb0 = gb_mm(0)
            vw_tt(0, gb0)
            psA[WAVE + 2] = mpsum.tile(
                [128, OCW], F32, name=f"ps0_{WAVE + 2}", tag="ps"
            )
            open_group_part(psA[WAVE + 2], wts[0], WAVE + 2, range(0, 8))
            gb1 = gb_mm(1)
            vw_tt(1, gb1)
            open_group_part(psA[WAVE + 2], wts[0], WAVE + 2, range(8, DC))

            # --- Close: fused LoRA-up + bias matmul, cast to bf16, DMA out.
            def close_group(ps, oc, t, last=False):
                nc.tensor.matmul(
                    ps,
                    lhsT=vwtr[:, t * 128 : (t + 1) * 128],
                    rhs=bcatr[:, oc * OCW : (oc + 1) * OCW],
                    start=False,
                    stop=True,
                )
                ot = opool.tile([128, OCW], BF16, tag="ot")
                if last:
                    # Pipeline the final tile's cast+DMA in quarters (split
                    # across DVE and ACT) so the tail isn't one serial
                    # cast -> DMA chain.
                    q = OCW // 4
                    for i in range(4):
                        sl = slice(i * q, (i + 1) * q)
                        if i % 2 == 0:
                            nc.vector.tensor_copy(ot[:, sl], ps[:, sl])
                        else:
                            nc.scalar.activation(
                                ot[:, sl],
                                ps[:, sl],
                                mybir.ActivationFunctionType.Copy,
                            )
                        nc.sync.dma_start(
                            out=out_d[oc, t, :, sl], in_=ot[:, sl]
                        )
                else:
                    nc.vector.tensor_copy(ot, ps)
                    nc.sync.dma_start(out=out_d[oc, t], in_=ot)

            for t in range(WAVE + 3):
                close_group(psA[t], 0, t)
            for t in range(WAVE + 3, TC):
                close_group(open_group(wts[0], t, f"ps0_{t}"), 0, t)
            for oc in range(1, OC):
                for t in range(TC):
                    close_group(
                        open_group(wts[oc], t, f"ps{oc}_{t}"),
                        oc,
                        t,
                        last=(oc == OC - 1 and t == TC - 1),
                    )

    nc.compile()
    return nc


def _prep_inputs(x, base_w, base_b, A, B, router_w):
    """Host-side layout prep: per-partition-contiguous bf16 DMA images."""
    import ml_dtypes

    bf16 = ml_dtypes.bfloat16

    x2 = np.ascontiguousarray(x, dtype=np.float32).reshape(TOK, D)
    # xt[core][p, dc*TPC + t] = x2[core*TPC + t, dc*128 + p]
    xv = x2.reshape(N_CORES, TPC, DC, 128)
    xt = (
        np.ascontiguousarray(xv.transpose(0, 3, 2, 1))
        .reshape(N_CORES, 128, DC * TPC)
        .astype(bf16)
    )

    # wt[oc, p, dc*OCW + o] = base_w[oc*OCW + o, dc*128 + p]
    wv = np.ascontiguousarray(base_w, dtype=np.float32).reshape(OC, OCW, DC, 128)
    wt = (
        np.ascontiguousarray(wv.transpose(0, 3, 2, 1))
        .reshape(OC, 128, DC * OCW)
        .astype(bf16)
    )

    # W1 = [A flattened to 32 rows; router_w 4 rows] over D
    W1 = np.concatenate(
        [
            np.asarray(A, dtype=np.float32).reshape(ER, D),
            np.asarray(router_w, np.float32),
        ],
        axis=0,
    )  # [36, D]
    w1t = (
        np.ascontiguousarray(W1.reshape(J, DC, 128).transpose(2, 1, 0))
        .reshape(128, DC * J)
        .astype(bf16)
    )

    # bcat rows 0..31: B[e, o, r] -> [er, o]; row 32: base_b; rows 33..127
    # zero padding (close matmul runs with a full 128-row stationary tile).
    bc = np.zeros((KP, O), np.float32)
    bc[0:ER] = np.asarray(B, dtype=np.float32).transpose(0, 2, 1).reshape(ER, O)
    bc[ER] = np.asarray(base_b, dtype=np.float32)
    bc = bc.astype(bf16)  # [128, O]

    # cst[:, :32] = per-expert expansion (E8); cst[:, 32:36] = 1/SCALE ones
    # block so the sums matmul replicates s/SCALE onto 4 rows and the recip
    # gives SCALE/s directly.
    cst = np.zeros((E, ER + 4), np.float32)
    for e in range(E):
        cst[e, e * R : (e + 1) * R] = 1.0
    cst[:, ER : ER + 4] = 1.0 / SCALE
    cst = cst.astype(bf16)

    return xt, wt, w1t, bc, cst


def kernel(x, base_w, base_b, A, B, router_w):
    global _cached
    if _cached is None:
        _cached = _build_program()
    nc = _cached

    xt, wt, w1t, bc, cst = _prep_inputs(x, base_w, base_b, A, B, router_w)

    in_maps = [
        {"xt": xt[c], "wt": wt, "w1t": w1t, "bcat": bc, "cst": cst}
        for c in range(N_CORES)
    ]
    core_ids = list(range(N_CORES))

    profile = os.environ.get("KERNEL_PROFILE", "0") == "1"
    res = run_bass_kernel_spmd(nc, in_maps, core_ids, trace=profile)

    last_run_info.clear()
    last_run_info["exec_time_ns"] = res.exec_time_ns
    last_run_info["mean_exec_time_ns"] = res.mean_exec_time_ns
    last_run_info["instructions_and_trace"] = res.instructions_and_trace
    last_run_info["profile_json"] = res.profile_json

    # out[core] shape [OC, TC, 128, OCW] bf16 -> tokens x features fp32
    full = np.empty((TOK, O), dtype=np.float32)
    for c in range(N_CORES):
        buf = res.results[c]["out"].astype(np.float32)  # [OC, TC, 128, OCW]
        full[c * TPC : (c + 1) * TPC] = buf.transpose(1, 2, 0, 3).reshape(TPC, O)
    return full.reshape(4, 2048, 2048)


# revision 30
# speedup vs baseline: 1.0195x; 1.0195x over previous
"""MoELoRALinear Trainium2 kernel (8-core data-parallel, Bass/Tile).

Math (per token t, out feature o):
    out[t,o] = x[t,:] @ base_w[o,:] + base_b[o]
             + sum_e softmax_e(x[t,:] @ router_w[e,:]) * SCALE
               * sum_r (x[t,:] @ A[e,r,:]) * B[e,o,r]

Strategy:
  - 8192 tokens sharded 8 ways (1024 tokens/core); weights replicated.
  - All operands shipped as bf16 (half the DMA bytes of fp32r; same PE
    throughput: 1 col/cycle). Output returned bf16, upcast on host.
    Norm rel-err ~2e-3, well inside the 2e-2 gate.
  - Per 128-col chunk: YT[j, tok] = [A;router].T @ x accumulated over all
    16 K-chunks (wide-N matmuls), interleaved with a 4-token-chunk base
    matmul wave (4+2 PSUM banks).
  - Softmax/gating WITHOUT PE transposes: exp on ACT straight from PSUM
    (no max-sub needed: logits are ~N(0,1)); per-token sums / broadcast
    done with tiny matmuls (ones / expansion-matrix lhsT); division and
    gating on DVE. The gated rank-32 projection lands directly in the
    [33, tok] lhsT layout the close matmuls need.
  - Close matmul (K=33: gated proj + ones row for bias) accumulates into
    the same PSUM group as the 16 base K-chunks.
  - DMA rings split: scalar=weights/tables, gpsimd=x stream, sync=outputs.
  - 4 warmup matmuls on a zero tile ramp the PE DVFS p-state during the
    fixed ~7us NEFF prologue + first x-chunk DMA.
"""

import os

import numpy as np

import concourse.bacc as bacc
import concourse.bass as bass
import concourse.mybir as mybir
from concourse.bass_utils import run_bass_kernel_spmd
from concourse.tile import TileContext

SCALE = 16.0 / 8.0  # alpha / r

N_CORES = 8
TOK = 8192  # 4 * 2048 tokens total
TPC = TOK // N_CORES  # tokens per core = 1024
D = 2048  # in features
O = 2048  # out features
E = 4
R = 8
ER = E * R  # 32
J = ER + E  # 36: rank-proj rows + router rows
DC = D // 128  # 16 contraction chunks
OCW = 512  # out-feature chunk width (one PSUM bank)
OC = O // OCW  # 4
TC = TPC // 128  # 8 token chunks per core
WAVE = 4  # wave-A token chunks (PSUM: WAVE + ytps1 + 2 filler groups + 1 ypsum)
KP = 128  # close-matmul contraction rows (33 used, padded to full array)

F32 = mybir.dt.float32
BF16 = mybir.dt.bfloat16

# Results of the last device run (for test harness inspection).
last_run_info: dict = {}

_cached = None


def _build_program():
    nc = bacc.Bacc()

    xt_d = nc.declare_dram_parameter("xt", [128, DC * TPC], BF16, isOutput=False)
    wt_d = nc.declare_dram_parameter("wt", [OC, 128, DC * OCW], BF16, isOutput=False)
    w1t_d = nc.declare_dram_parameter("w1t", [128, DC * J], BF16, isOutput=False)
    bcat_d = nc.declare_dram_parameter("bcat", [KP, O], BF16, isOutput=False)
    cst_d = nc.declare_dram_parameter("cst", [E, ER + 4], BF16, isOutput=False)
    out_d = nc.declare_dram_parameter("out", [OC, TC, 128, OCW], BF16, isOutput=True)

    MUL = mybir.AluOpType.mult

    with TileContext(nc) as tc:
        with (
            tc.tile_pool(name="cpool", bufs=1) as cpool,
            tc.tile_pool(name="wpool", bufs=4) as wpool,
            tc.tile_pool(name="opool", bufs=6) as opool,
            tc.tile_pool(name="mpsum", bufs=7, space="PSUM") as mpsum,
            tc.tile_pool(name="ypsum", bufs=1, space="PSUM") as ypsum,
        ):
            # x gets a dedicated ring (gpsimd) so its stream isn't halved by
            # the weight traffic; everything else rides the scalar ring in
            # consumption order (tables, w0 pieces, then bcat/w1..w3 which
            # are not needed before ~60us). Outputs go on sync.
            cstr = cpool.tile([E, ER + 4], BF16)
            nc.scalar.dma_start(out=cstr, in_=cst_d[:, :])
            w1tr = cpool.tile([128, DC * J], BF16)
            nc.scalar.dma_start(out=w1tr, in_=w1t_d[:, :])

            wts = {
                oc: wpool.tile([128, DC * OCW], BF16, name=f"wtr{oc}", tag="wtr")
                for oc in range(OC)
            }
            # w0 in 4 k-block pieces (wave A unlocks per piece).
            for k in range(4):
                nc.scalar.dma_start(
                    out=wts[0][:, k * 4 * OCW : (k + 1) * 4 * OCW],
                    in_=wt_d[0, :, k * 4 * OCW : (k + 1) * 4 * OCW],
                )
            bcatr = cpool.tile([KP, O], BF16)
            nc.scalar.dma_start(out=bcatr, in_=bcat_d[:, :])
            for oc in range(1, OC):
                for h in range(2):
                    nc.scalar.dma_start(
                        out=wts[oc][:, h * 8 * OCW : (h + 1) * 8 * OCW],
                        in_=wt_d[oc, :, h * 8 * OCW : (h + 1) * 8 * OCW],
                    )

            # x split across two rings with DISJOINT tiles (one tile written
            # by two DMA queues hangs the DGE): dc0-11 fine-grained on the
            # fast sync ring, dc12-15 on the otherwise-idle gpsimd ring.
            XSPL = 12
            xa = cpool.tile([128, XSPL * TPC], BF16)
            xb = cpool.tile([128, (DC - XSPL) * TPC], BF16)

            def xsl(dc, a, b):
                if dc < XSPL:
                    return xa[:, dc * TPC + a : dc * TPC + b]
                return xb[:, (dc - XSPL) * TPC + a : (dc - XSPL) * TPC + b]

            def load_x(eng, tile, t0, d0, nd):
                eng.dma_start(
                    out=tile[:, (d0 - t0) * TPC : (d0 - t0 + nd) * TPC],
                    in_=xt_d[:, d0 * TPC : (d0 + nd) * TPC],
                )

            load_x(nc.gpsimd, xb, XSPL, 12, 2)
            load_x(nc.gpsimd, xb, XSPL, 14, 2)
            for dc in range(4):
                load_x(nc.sync, xa, 0, dc, 1)
            for dp in range(2, 6):
                load_x(nc.sync, xa, 0, 2 * dp, 2)

            # Warmup: ramp the PE p-state on a zero tile while the first
            # x chunk is still in flight.
            warm_sb = cpool.tile([128, OCW], BF16)
            nc.vector.memset(warm_sb, 0.0)
            # Close lhsT padded to 128 contraction rows: 32 gated-proj rows,
            # the ones row (bias), then zeros (avoids 64-row PE tile mode).
            vwtr = cpool.tile([KP, TPC], BF16)
            for p0 in range(ER, KP, 32):
                nc.vector.memset(vwtr[p0 : p0 + 32, :], 0.0)
            nc.vector.memset(vwtr[ER : ER + 1, :], 1.0)
            warmps = ypsum.tile([128, OCW], F32, name="warmps", tag="yb")
            for _ in range(8):
                nc.tensor.matmul(
                    warmps, lhsT=warm_sb[:, 0:128], rhs=warm_sb, start=True, stop=True
                )

            # --- Router/LoRA-down phase: YT[j, tok] accumulated over all
            # dc, interleaved with wave-A base matmuls in DMA arrival order.
            # ytps[1] lives in the mpsum pool so ypsum (bufs=1) can rotate
            # the small gating-chain tiles behind it.
            ytps = [
                ypsum.tile([J, 512], F32, name="ytps0", tag="yb"),
                mpsum.tile([J, 512], F32, name="ytps1", tag="ps"),
            ]
            psA = {
                t: mpsum.tile([128, OCW], F32, name=f"ps0_{t}", tag="ps")
                for t in range(WAVE)
            }
            for k in range(4):
                for dc in range(4 * k, 4 * k + 4):
                    for th in range(2):
                        nc.tensor.matmul(
                            ytps[th],
                            lhsT=w1tr[:, dc * J : (dc + 1) * J],
                            rhs=xsl(dc, th * 512, (th + 1) * 512),
                            start=(dc == 0),
                            stop=(dc == DC - 1),
                        )
                for dc in range(4 * k, 4 * k + 4):
                    for t in range(WAVE):
                        nc.tensor.matmul(
                            psA[t],
                            lhsT=xsl(dc, t * 128, (t + 1) * 128),
                            rhs=wts[0][:, dc * OCW : (dc + 1) * OCW],
                            start=(dc == 0),
                            stop=False,
                        )

            # --- Gating chain (no PE transposes).
            # u = exp(logits) straight off PSUM (no max-sub: logits ~N(0,1));
            # sums4 = 0.5-ones matmul replicates s/SCALE onto 4 rows;
            # r4 = SCALE/s via fast approx recip; g = u*r4 on DVE;
            # gb = E8.T@g broadcasts per-expert gates to the 32 rank rows;
            # vw = Y * gb lands directly in the [32, tok] bf16 close layout.
            y_sb = cpool.tile([ER, TPC], BF16)
            u_sb = cpool.tile([E, TPC], BF16)
            r_sb = cpool.tile([E, TPC], F32)
            g_sb = cpool.tile([E, TPC], BF16)
            for th in range(2):
                nc.vector.tensor_copy(
                    y_sb[:, th * 512 : (th + 1) * 512], ytps[th][0:ER, :]
                )
                nc.scalar.activation(
                    u_sb[:, th * 512 : (th + 1) * 512],
                    ytps[th][ER:J, :],
                    mybir.ActivationFunctionType.Exp,
                )

            def open_group(wtr, t, name):
                ps = mpsum.tile([128, OCW], F32, name=name, tag="ps")
                for dc in range(DC):
                    nc.tensor.matmul(
                        ps,
                        lhsT=xsl(dc, t * 128, (t + 1) * 128),
                        rhs=wtr[:, dc * OCW : (dc + 1) * OCW],
                        start=(dc == 0),
                        stop=False,
                    )
                return ps

            def open_group_part(ps, wtr, t, dcs):
                for dc in dcs:
                    nc.tensor.matmul(
                        ps,
                        lhsT=xsl(dc, t * 128, (t + 1) * 128),
                        rhs=wtr[:, dc * OCW : (dc + 1) * OCW],
                        start=(dc == 0),
                        stop=False,
                    )

            # The t4/t5/t6 base groups need no new DMA data, so their 48
            # matmuls are interleaved as PE filler around the chain's small
            # matmuls; the single-buf ypsum rotation serializes the chain
            # (sums0 -> recip0 -> sums1 -> recip1 -> gb0 -> vw0 -> gb1 ->
            # vw1), each hop hidden behind ~1.7us of base matmuls.
            def sums_mm(th):
                s = ypsum.tile([E, 512], F32, name=f"sums{th}", tag="yb")
                nc.tensor.matmul(
                    s,
                    lhsT=cstr[0:E, ER : ER + 4],
                    rhs=u_sb[:, th * 512 : (th + 1) * 512],
                    start=True,
                    stop=True,
                )
                return s

            def recip_g(th, s):
                nc.vector.reciprocal_approx_fast(
                    out=r_sb[:, th * 512 : (th + 1) * 512], in_=s
                )
                nc.vector.tensor_tensor(
                    g_sb[:, th * 512 : (th + 1) * 512],
                    u_sb[:, th * 512 : (th + 1) * 512],
                    r_sb[:, th * 512 : (th + 1) * 512],
                    op=MUL,
                )

            def gb_mm(th):
                gb = ypsum.tile([ER, 512], F32, name=f"gb{th}", tag="yb")
                nc.tensor.matmul(
                    gb,
                    lhsT=cstr[0:E, 0:ER],
                    rhs=g_sb[:, th * 512 : (th + 1) * 512],
                    start=True,
                    stop=True,
                )
                return gb

            def vw_tt(th, gb):
                nc.vector.tensor_tensor(
                    vwtr[0:ER, th * 512 : (th + 1) * 512],
                    y_sb[:, th * 512 : (th + 1) * 512],
                    gb,
                    op=MUL,
                )

            psA[WAVE] = open_group(wts[0], WAVE, f"ps0_{WAVE}")
            s0 = sums_mm(0)
            recip_g(0, s0)
            psA[WAVE + 1] = mpsum.tile(
                [128, OCW], F32, name=f"ps0_{WAVE + 1}", tag="ps"
            )
            open_group_part(psA[WAVE + 1], wts[0], WAVE + 1, range(0, 8))
            s1 = sums_mm(1)
            recip_g(1, s1)
            open_group_part(psA[WAVE + 1], wts[0], WAVE + 1, range(8, DC))
            gb0 = gb_mm(0)
            vw_tt(0, gb0)
            psA[WAVE + 2] = mpsum.tile(
                [128, OCW], F32, name=f"ps0_{WAVE + 2}", tag="ps"
            )
            open_group_part(psA[WAVE + 2], wts[0], WAVE + 2, range(0, 8))
            gb1 = gb_mm(1)
            vw_tt(1, gb1)
            open_group_part(psA[WAVE + 2], wts[0], WAVE + 2, range(8, DC))

            # --- Close: fused LoRA-up + bias matmul, cast to bf16, DMA out.
            def close_group(ps, oc, t, last=False):
                nc.tensor.matmul(
                    ps,
                    lhsT=vwtr[:, t * 128 : (t + 1) * 128],
                    rhs=bcatr[:, oc * OCW : (oc + 1) * OCW],
                    start=False,
                    stop=True,
                )
                ot = opool.tile([128, OCW], BF16, tag="ot")
                if last:
                    # Pipeline the final tile's cast+DMA in halves (DVE and
                    # ACT in parallel) so the tail isn't one serial
                    # cast -> DMA chain; more splits lose to the ~0.6us
                    # per-DMA issue cost.
                    h = OCW // 2
                    nc.vector.tensor_copy(ot[:, 0:h], ps[:, 0:h])
                    nc.scalar.activation(
                        ot[:, h:OCW],
                        ps[:, h:OCW],
                        mybir.ActivationFunctionType.Copy,
                    )
                    nc.sync.dma_start(out=out_d[oc, t, :, 0:h], in_=ot[:, 0:h])
                    nc.sync.dma_start(
                        out=out_d[oc, t, :, h:OCW], in_=ot[:, h:OCW]
                    )
                else:
                    nc.vector.tensor_copy(ot, ps)
                    nc.sync.dma_start(out=out_d[oc, t], in_=ot)

            for t in range(WAVE + 3):
                close_group(psA[t], 0, t)
            for t in range(WAVE + 3, TC):
                close_group(open_group(wts[0], t, f"ps0_{t}"), 0, t)
            for oc in range(1, OC):
                for t in range(TC):
                    close_group(
                        open_group(wts[oc], t, f"ps{oc}_{t}"),
                        oc,
                        t,
                        last=(oc == OC - 1 and t == TC - 1),
                    )

    nc.compile()
    return nc


def _prep_inputs(x, base_w, base_b, A, B, router_w):
    """Host-side layout prep: per-partition-contiguous bf16 DMA images."""
    import ml_dtypes

    bf16 = ml_dtypes.bfloat16

    x2 = np.ascontiguousarray(x, dtype=np.float32).reshape(TOK, D)
    # xt[core][p, dc*TPC + t] = x2[core*TPC + t, dc*128 + p]
    xv = x2.reshape(N_CORES, TPC, DC, 128)
    xt = (
        np.ascontiguousarray(xv.transpose(0, 3, 2, 1))
        .reshape(N_CORES, 128, DC * TPC)
        .astype(bf16)
    )

    # wt[oc, p, dc*OCW + o] = base_w[oc*OCW + o, dc*128 + p]
    wv = np.ascontiguousarray(base_w, dtype=np.float32).reshape(OC, OCW, DC, 128)
    wt = (
        np.ascontiguousarray(wv.transpose(0, 3, 2, 1))
        .reshape(OC, 128, DC * OCW)
        .astype(bf16)
    )

    # W1 = [A flattened to 32 rows; router_w 4 rows] over D
    W1 = np.concatenate(
        [
            np.asarray(A, dtype=np.float32).reshape(ER, D),
            np.asarray(router_w, np.float32),
        ],
        axis=0,
    )  # [36, D]
    w1t = (
        np.ascontiguousarray(W1.reshape(J, DC, 128).transpose(2, 1, 0))
        .reshape(128, DC * J)
        .astype(bf16)
    )

    # bcat rows 0..31: B[e, o, r] -> [er, o]; row 32: base_b; rows 33..127
    # zero padding (close matmul runs with a full 128-row stationary tile).
    bc = np.zeros((KP, O), np.float32)
    bc[0:ER] = np.asarray(B, dtype=np.float32).transpose(0, 2, 1).reshape(ER, O)
    bc[ER] = np.asarray(base_b, dtype=np.float32)
    bc = bc.astype(bf16)  # [128, O]

    # cst[:, :32] = per-expert expansion (E8); cst[:, 32:36] = 1/SCALE ones
    # block so the sums matmul replicates s/SCALE onto 4 rows and the recip
    # gives SCALE/s directly.
    cst = np.zeros((E, ER + 4), np.float32)
    for e in range(E):
        cst[e, e * R : (e + 1) * R] = 1.0
    cst[:, ER : ER + 4] = 1.0 / SCALE
    cst = cst.astype(bf16)

    return xt, wt, w1t, bc, cst


def kernel(x, base_w, base_b, A, B, router_w):
    global _cached
    if _cached is None:
        _cached = _build_program()
    nc = _cached

    xt, wt, w1t, bc, cst = _prep_inputs(x, base_w, base_b, A, B, router_w)

    in_maps = [
        {"xt": xt[c], "wt": wt, "w1t": w1t, "bcat": bc, "cst": cst}
        for c in range(N_CORES)
    ]
    core_ids = list(range(N_CORES))

    profile = os.environ.get("KERNEL_PROFILE", "0") == "1"
    res = run_bass_kernel_spmd(nc, in_maps, core_ids, trace=profile)

    last_run_info.clear()
    last_run_info["exec_time_ns"] = res.exec_time_ns
    last_run_info["mean_exec_time_ns"] = res.mean_exec_time_ns
    last_run_info["instructions_and_trace"] = res.instructions_and_trace
    last_run_info["profile_json"] = res.profile_json

    # out[core] shape [OC, TC, 128, OCW] bf16 -> tokens x features fp32
    full = np.empty((TOK, O), dtype=np.float32)
    for c in range(N_CORES):
        buf = res.results[c]["out"].astype(np.float32)  # [OC, TC, 128, OCW]
        full[c * TPC : (c + 1) * TPC] = buf.transpose(1, 2, 0, 3).reshape(TPC, O)
    return full.reshape(4, 2048, 2048)


# revision 32
# speedup vs baseline: 1.0231x; 1.0036x over previous
"""MoELoRALinear Trainium2 kernel (8-core data-parallel, Bass/Tile).

Math (per token t, out feature o):
    out[t,o] = x[t,:] @ base_w[o,:] + base_b[o]
             + sum_e softmax_e(x[t,:] @ router_w[e,:]) * SCALE
               * sum_r (x[t,:] @ A[e,r,:]) * B[e,o,r]

Strategy:
  - 8192 tokens sharded 8 ways (1024 tokens/core); weights replicated.
  - All operands shipped as bf16 (half the DMA bytes of fp32r; same PE
    throughput: 1 col/cycle). Output returned bf16, upcast on host.
    Norm rel-err ~2e-3, well inside the 2e-2 gate.
  - Per 128-col chunk: YT[j, tok] = [A;router].T @ x accumulated over all
    16 K-chunks (wide-N matmuls), interleaved with a 4-token-chunk base
    matmul wave (4+2 PSUM banks).
  - Softmax/gating WITHOUT PE transposes: exp on ACT straight from PSUM
    (no max-sub needed: logits are ~N(0,1)); per-token sums / broadcast
    done with tiny matmuls (ones / expansion-matrix lhsT); division and
    gating on DVE. The gated rank-32 projection lands directly in the
    [33, tok] lhsT layout the close matmuls need.
  - Close matmul (K=33: gated proj + ones row for bias) accumulates into
    the same PSUM group as the 16 base K-chunks.
  - DMA rings split: scalar=weights/tables, gpsimd=x stream, sync=outputs.
  - 4 warmup matmuls on a zero tile ramp the PE DVFS p-state during the
    fixed ~7us NEFF prologue + first x-chunk DMA.
"""

import os

import numpy as np

import concourse.bacc as bacc
import concourse.bass as bass
import concourse.mybir as mybir
from concourse.bass_utils import run_bass_kernel_spmd
from concourse.tile import TileContext

SCALE = 16.0 / 8.0  # alpha / r

N_CORES = 8
TOK = 8192  # 4 * 2048 tokens total
TPC = TOK // N_CORES  # tokens per core = 1024
D = 2048  # in features
O = 2048  # out features
E = 4
R = 8
ER = E * R  # 32
J = ER + E  # 36: rank-proj rows + router rows
DC = D // 128  # 16 contraction chunks
OCW = 512  # out-feature chunk width (one PSUM bank)
OC = O // OCW  # 4
TC = TPC // 128  # 8 token chunks per core
WAVE = 4  # wave-A token chunks (PSUM: WAVE + ytps1 + 2 filler groups + 1 ypsum)
KP = 128  # close-matmul contraction rows (33 used, padded to full array)

F32 = mybir.dt.float32
BF16 = mybir.dt.bfloat16

# Results of the last device run (for test harness inspection).
last_run_info: dict = {}

_cached = None


def _build_program():
    nc = bacc.Bacc()

    xt_d = nc.declare_dram_parameter("xt", [128, DC * TPC], BF16, isOutput=False)
    wt_d = nc.declare_dram_parameter("wt", [OC, 128, DC * OCW], BF16, isOutput=False)
    w1t_d = nc.declare_dram_parameter("w1t", [128, DC * J], BF16, isOutput=False)
    bcat_d = nc.declare_dram_parameter("bcat", [KP, O], BF16, isOutput=False)
    cst_d = nc.declare_dram_parameter("cst", [E, ER + 4], BF16, isOutput=False)
    out_d = nc.declare_dram_parameter("out", [OC, TC, 128, OCW], BF16, isOutput=True)

    MUL = mybir.AluOpType.mult

    with TileContext(nc) as tc:
        with (
            tc.tile_pool(name="cpool", bufs=1) as cpool,
            tc.tile_pool(name="wpool", bufs=4) as wpool,
            tc.tile_pool(name="opool", bufs=6) as opool,
            tc.tile_pool(name="mpsum", bufs=7, space="PSUM") as mpsum,
            tc.tile_pool(name="ypsum", bufs=1, space="PSUM") as ypsum,
        ):
            # x gets a dedicated ring (gpsimd) so its stream isn't halved by
            # the weight traffic; everything else rides the scalar ring in
            # consumption order (tables, w0 pieces, then bcat/w1..w3 which
            # are not needed before ~60us). Outputs go on sync.
            cstr = cpool.tile([E, ER + 4], BF16)
            nc.scalar.dma_start(out=cstr, in_=cst_d[:, :])
            w1tr = cpool.tile([128, DC * J], BF16)
            nc.scalar.dma_start(out=w1tr, in_=w1t_d[:, :])

            wts = {
                oc: wpool.tile([128, DC * OCW], BF16, name=f"wtr{oc}", tag="wtr")
                for oc in range(OC)
            }
            # w0 in 4 k-block pieces (wave A unlocks per piece).
            for k in range(4):
                nc.scalar.dma_start(
                    out=wts[0][:, k * 4 * OCW : (k + 1) * 4 * OCW],
                    in_=wt_d[0, :, k * 4 * OCW : (k + 1) * 4 * OCW],
                )
            bcatr = cpool.tile([KP, O], BF16)
            nc.scalar.dma_start(out=bcatr, in_=bcat_d[:, :])
            for oc in range(1, OC):
                for h in range(2):
                    nc.scalar.dma_start(
                        out=wts[oc][:, h * 8 * OCW : (h + 1) * 8 * OCW],
                        in_=wt_d[oc, :, h * 8 * OCW : (h + 1) * 8 * OCW],
                    )

            # x split across two rings with DISJOINT tiles per ring (one
            # tile written by two DMA queues hangs the DGE). The first four
            # chunks alternate rings so the PE's per-chunk ladder never
            # waits on a single serial stream; the gpsimd ring also takes
            # the last four chunks.
            XSPL = 12
            xa = cpool.tile([128, XSPL * TPC], BF16)  # sync: dc0,2,4..11
            xg = cpool.tile([128, 2 * TPC], BF16)  # gpsimd: dc1, dc3
            xb = cpool.tile([128, (DC - XSPL) * TPC], BF16)  # gpsimd: dc12-15

            def xsl(dc, a, b):
                if dc == 1 or dc == 3:
                    return xg[:, (dc // 2) * TPC + a : (dc // 2) * TPC + b]
                if dc < XSPL:
                    return xa[:, dc * TPC + a : dc * TPC + b]
                return xb[:, (dc - XSPL) * TPC + a : (dc - XSPL) * TPC + b]

            def load_x(eng, out_ap, d0, nd):
                eng.dma_start(
                    out=out_ap, in_=xt_d[:, d0 * TPC : (d0 + nd) * TPC]
                )

            load_x(nc.sync, xa[:, 0:TPC], 0, 1)
            load_x(nc.gpsimd, xg[:, 0:TPC], 1, 1)
            load_x(nc.sync, xa[:, 2 * TPC : 3 * TPC], 2, 1)
            load_x(nc.gpsimd, xg[:, TPC : 2 * TPC], 3, 1)
            for dp in range(2, 6):
                load_x(
                    nc.sync, xa[:, 2 * dp * TPC : (2 * dp + 2) * TPC], 2 * dp, 2
                )
            load_x(nc.gpsimd, xb[:, 0 : 2 * TPC], 12, 2)
            load_x(nc.gpsimd, xb[:, 2 * TPC : 4 * TPC], 14, 2)

            # Warmup: ramp the PE p-state on a zero tile while the first
            # x chunk is still in flight.
            warm_sb = cpool.tile([128, OCW], BF16)
            nc.vector.memset(warm_sb, 0.0)
            # Close lhsT padded to 128 contraction rows: 32 gated-proj rows,
            # the ones row (bias), then zeros (avoids 64-row PE tile mode).
            vwtr = cpool.tile([KP, TPC], BF16)
            for p0 in range(ER, KP, 32):
                nc.vector.memset(vwtr[p0 : p0 + 32, :], 0.0)
            nc.vector.memset(vwtr[ER : ER + 1, :], 1.0)
            warmps = ypsum.tile([128, OCW], F32, name="warmps", tag="yb")
            for _ in range(6):
                nc.tensor.matmul(
                    warmps, lhsT=warm_sb[:, 0:128], rhs=warm_sb, start=True, stop=True
                )

            # --- Router/LoRA-down phase: YT[j, tok] accumulated over all
            # dc, interleaved with wave-A base matmuls in DMA arrival order.
            # ytps[1] lives in the mpsum pool so ypsum (bufs=1) can rotate
            # the small gating-chain tiles behind it.
            ytps = [
                ypsum.tile([J, 512], F32, name="ytps0", tag="yb"),
                mpsum.tile([J, 512], F32, name="ytps1", tag="ps"),
            ]
            psA = {
                t: mpsum.tile([128, OCW], F32, name=f"ps0_{t}", tag="ps")
                for t in range(WAVE)
            }
            for k in range(4):
                for dc in range(4 * k, 4 * k + 4):
                    for th in range(2):
                        nc.tensor.matmul(
                            ytps[th],
                            lhsT=w1tr[:, dc * J : (dc + 1) * J],
                            rhs=xsl(dc, th * 512, (th + 1) * 512),
                            start=(dc == 0),
                            stop=(dc == DC - 1),
                        )
                for dc in range(4 * k, 4 * k + 4):
                    for t in range(WAVE):
                        nc.tensor.matmul(
                            psA[t],
                            lhsT=xsl(dc, t * 128, (t + 1) * 128),
                            rhs=wts[0][:, dc * OCW : (dc + 1) * OCW],
                            start=(dc == 0),
                            stop=False,
                        )

            # --- Gating chain (no PE transposes).
            # u = exp(logits) straight off PSUM (no max-sub: logits ~N(0,1));
            # sums4 = 0.5-ones matmul replicates s/SCALE onto 4 rows;
            # r4 = SCALE/s via fast approx recip; g = u*r4 on DVE;
            # gb = E8.T@g broadcasts per-expert gates to the 32 rank rows;
            # vw = Y * gb lands directly in the [32, tok] bf16 close layout.
            y_sb = cpool.tile([ER, TPC], BF16)
            u_sb = cpool.tile([E, TPC], BF16)
            r_sb = cpool.tile([E, TPC], F32)
            g_sb = cpool.tile([E, TPC], BF16)
            for th in range(2):
                nc.vector.tensor_copy(
                    y_sb[:, th * 512 : (th + 1) * 512], ytps[th][0:ER, :]
                )
                nc.scalar.activation(
                    u_sb[:, th * 512 : (th + 1) * 512],
                    ytps[th][ER:J, :],
                    mybir.ActivationFunctionType.Exp,
                )

            def open_group(wtr, t, name):
                ps = mpsum.tile([128, OCW], F32, name=name, tag="ps")
                for dc in range(DC):
                    nc.tensor.matmul(
                        ps,
                        lhsT=xsl(dc, t * 128, (t + 1) * 128),
                        rhs=wtr[:, dc * OCW : (dc + 1) * OCW],
                        start=(dc == 0),
                        stop=False,
                    )
                return ps

            def open_group_part(ps, wtr, t, dcs):
                for dc in dcs:
                    nc.tensor.matmul(
                        ps,
                        lhsT=xsl(dc, t * 128, (t + 1) * 128),
                        rhs=wtr[:, dc * OCW : (dc + 1) * OCW],
                        start=(dc == 0),
                        stop=False,
                    )

            # The t4/t5/t6 base groups need no new DMA data, so their 48
            # matmuls are interleaved as PE filler around the chain's small
            # matmuls; the single-buf ypsum rotation serializes the chain
            # (sums0 -> recip0 -> sums1 -> recip1 -> gb0 -> vw0 -> gb1 ->
            # vw1), each hop hidden behind ~1.7us of base matmuls.
            def sums_mm(th):
                s = ypsum.tile([E, 512], F32, name=f"sums{th}", tag="yb")
                nc.tensor.matmul(
                    s,
                    lhsT=cstr[0:E, ER : ER + 4],
                    rhs=u_sb[:, th * 512 : (th + 1) * 512],
                    start=True,
                    stop=True,
                )
                return s

            def recip_g(th, s):
                nc.vector.reciprocal_approx_fast(
                    out=r_sb[:, th * 512 : (th + 1) * 512], in_=s
                )
                nc.vector.tensor_tensor(
                    g_sb[:, th * 512 : (th + 1) * 512],
                    u_sb[:, th * 512 : (th + 1) * 512],
                    r_sb[:, th * 512 : (th + 1) * 512],
                    op=MUL,
                )

            def gb_mm(th):
                gb = ypsum.tile([ER, 512], F32, name=f"gb{th}", tag="yb")
                nc.tensor.matmul(
                    gb,
                    lhsT=cstr[0:E, 0:ER],
                    rhs=g_sb[:, th * 512 : (th + 1) * 512],
                    start=True,
                    stop=True,
                )
                return gb

            def vw_tt(th, gb):
                nc.vector.tensor_tensor(
                    vwtr[0:ER, th * 512 : (th + 1) * 512],
                    y_sb[:, th * 512 : (th + 1) * 512],
                    gb,
                    op=MUL,
                )

            psA[WAVE] = open_group(wts[0], WAVE, f"ps0_{WAVE}")
            s0 = sums_mm(0)
            recip_g(0, s0)
            psA[WAVE + 1] = mpsum.tile(
                [128, OCW], F32, name=f"ps0_{WAVE + 1}", tag="ps"
            )
            open_group_part(psA[WAVE + 1], wts[0], WAVE + 1, range(0, 8))
            s1 = sums_mm(1)
            recip_g(1, s1)
            open_group_part(psA[WAVE + 1], wts[0], WAVE + 1, range(8, DC))
            gb0 = gb_mm(0)
            vw_tt(0, gb0)
            psA[WAVE + 2] = mpsum.tile(
                [128, OCW], F32, name=f"ps0_{WAVE + 2}", tag="ps"
            )
            open_group_part(psA[WAVE + 2], wts[0], WAVE + 2, range(0, 8))
            gb1 = gb_mm(1)
            vw_tt(1, gb1)
            open_group_part(psA[WAVE + 2], wts[0], WAVE + 2, range(8, DC))

            # --- Close: fused LoRA-up + bias matmul, cast to bf16, DMA out.
            def close_group(ps, oc, t, last=False):
                nc.tensor.matmul(
                    ps,
                    lhsT=vwtr[:, t * 128 : (t + 1) * 128],
                    rhs=bcatr[:, oc * OCW : (oc + 1) * OCW],
                    start=False,
                    stop=True,
                )
                ot = opool.tile([128, OCW], BF16, tag="ot")
                if last:
                    # Pipeline the final tile's cast+DMA in halves (DVE and
                    # ACT in parallel) so the tail isn't one serial
                    # cast -> DMA chain; more splits lose to the ~0.6us
                    # per-DMA issue cost.
                    h = OCW // 2
                    nc.vector.tensor_copy(ot[:, 0:h], ps[:, 0:h])
                    nc.scalar.activation(
                        ot[:, h:OCW],
                        ps[:, h:OCW],
                        mybir.ActivationFunctionType.Copy,
                    )
                    nc.sync.dma_start(out=out_d[oc, t, :, 0:h], in_=ot[:, 0:h])
                    nc.sync.dma_start(
                        out=out_d[oc, t, :, h:OCW], in_=ot[:, h:OCW]
                    )
                else:
                    nc.vector.tensor_copy(ot, ps)
                    nc.sync.dma_start(out=out_d[oc, t], in_=ot)

            for t in range(WAVE + 3):
                close_group(psA[t], 0, t)
            for t in range(WAVE + 3, TC):
                close_group(open_group(wts[0], t, f"ps0_{t}"), 0, t)
            for oc in range(1, OC):
                for t in range(TC):
                    close_group(
                        open_group(wts[oc], t, f"ps{oc}_{t}"),
                        oc,
                        t,
                        last=(oc == OC - 1 and t == TC - 1),
                    )

    nc.compile()
    return nc


def _prep_inputs(x, base_w, base_b, A, B, router_w):
    """Host-side layout prep: per-partition-contiguous bf16 DMA images."""
    import ml_dtypes

    bf16 = ml_dtypes.bfloat16

    x2 = np.ascontiguousarray(x, dtype=np.float32).reshape(TOK, D)
    # xt[core][p, dc*TPC + t] = x2[core*TPC + t, dc*128 + p]
    xv = x2.reshape(N_CORES, TPC, DC, 128)
    xt = (
        np.ascontiguousarray(xv.transpose(0, 3, 2, 1))
        .reshape(N_CORES, 128, DC * TPC)
        .astype(bf16)
    )

    # wt[oc, p, dc*OCW + o] = base_w[oc*OCW + o, dc*128 + p]
    wv = np.ascontiguousarray(base_w, dtype=np.float32).reshape(OC, OCW, DC, 128)
    wt = (
        np.ascontiguousarray(wv.transpose(0, 3, 2, 1))
        .reshape(OC, 128, DC * OCW)
        .astype(bf16)
    )

    # W1 = [A flattened to 32 rows; router_w 4 rows] over D
    W1 = np.concatenate(
        [
            np.asarray(A, dtype=np.float32).reshape(ER, D),
            np.asarray(router_w, np.float32),
        ],
        axis=0,
    )  # [36, D]
    w1t = (
        np.ascontiguousarray(W1.reshape(J, DC, 128).transpose(2, 1, 0))
        .reshape(128, DC * J)
        .astype(bf16)
    )

    # bcat rows 0..31: B[e, o, r] -> [er, o]; row 32: base_b; rows 33..127
    # zero padding (close matmul runs with a full 128-row stationary tile).
    bc = np.zeros((KP, O), np.float32)
    bc[0:ER] = np.asarray(B, dtype=np.float32).transpose(0, 2, 1).reshape(ER, O)
    bc[ER] = np.asarray(base_b, dtype=np.float32)
    bc = bc.astype(bf16)  # [128, O]

    # cst[:, :32] = per-expert expansion (E8); cst[:, 32:36] = 1/SCALE ones
    # block so the sums matmul replicates s/SCALE onto 4 rows and the recip
    # gives SCALE/s directly.
    cst = np.zeros((E, ER + 4), np.float32)
    for e in range(E):
        cst[e, e * R : (e + 1) * R] = 1.0
    cst[:, ER : ER + 4] = 1.0 / SCALE
    cst = cst.astype(bf16)

    return xt, wt, w1t, bc, cst


def kernel(x, base_w, base_b, A, B, router_w):
    global _cached
    if _cached is None:
        _cached = _build_program()
    nc = _cached

    xt, wt, w1t, bc, cst = _prep_inputs(x, base_w, base_b, A, B, router_w)

    in_maps = [
        {"xt": xt[c], "wt": wt, "w1t": w1t, "bcat": bc, "cst": cst}
        for c in range(N_CORES)
    ]
    core_ids = list(range(N_CORES))

    profile = os.environ.get("KERNEL_PROFILE", "0") == "1"
    res = run_bass_kernel_spmd(nc, in_maps, core_ids, trace=profile)

    last_run_info.clear()
    last_run_info["exec_time_ns"] = res.exec_time_ns
    last_run_info["mean_exec_time_ns"] = res.mean_exec_time_ns
    last_run_info["instructions_and_trace"] = res.instructions_and_trace
    last_run_info["profile_json"] = res.profile_json

    # out[core] shape [OC, TC, 128, OCW] bf16 -> tokens x features fp32
    full = np.empty((TOK, O), dtype=np.float32)
    for c in range(N_CORES):
        buf = res.results[c]["out"].astype(np.float32)  # [OC, TC, 128, OCW]
        full[c * TPC : (c + 1) * TPC] = buf.transpose(1, 2, 0, 3).reshape(TPC, O)
    return full.reshape(4, 2048, 2048)


# revision 34
# speedup vs baseline: 1.0429x; 1.0193x over previous
"""MoELoRALinear Trainium2 kernel (8-core data-parallel, Bass/Tile).

Math (per token t, out feature o):
    out[t,o] = x[t,:] @ base_w[o,:] + base_b[o]
             + sum_e softmax_e(x[t,:] @ router_w[e,:]) * SCALE
               * sum_r (x[t,:] @ A[e,r,:]) * B[e,o,r]

Strategy:
  - 8192 tokens sharded 8 ways (1024 tokens/core); weights replicated.
  - All operands shipped as bf16 (half the DMA bytes of fp32r; same PE
    throughput: 1 col/cycle). Output returned bf16, upcast on host.
    Norm rel-err ~2e-3, well inside the 2e-2 gate.
  - Per 128-col chunk: YT[j, tok] = [A;router].T @ x accumulated over all
    16 K-chunks (wide-N matmuls), interleaved with a 4-token-chunk base
    matmul wave (4+2 PSUM banks).
  - Softmax/gating WITHOUT PE transposes: exp on ACT straight from PSUM
    (no max-sub needed: logits are ~N(0,1)); per-token sums / broadcast
    done with tiny matmuls (ones / expansion-matrix lhsT); division and
    gating on DVE. The gated rank-32 projection lands directly in the
    [33, tok] lhsT layout the close matmuls need.
  - Close matmul (K=33: gated proj + ones row for bias) accumulates into
    the same PSUM group as the 16 base K-chunks.
  - DMA rings split: scalar=weights/tables, gpsimd=x stream, sync=outputs.
  - 4 warmup matmuls on a zero tile ramp the PE DVFS p-state during the
    fixed ~7us NEFF prologue + first x-chunk DMA.
"""

import os

import numpy as np

import concourse.bacc as bacc
import concourse.bass as bass
import concourse.mybir as mybir
from concourse.bass_utils import run_bass_kernel_spmd
from concourse.tile import TileContext

SCALE = 16.0 / 8.0  # alpha / r

N_CORES = 8
TOK = 8192  # 4 * 2048 tokens total
TPC = TOK // N_CORES  # tokens per core = 1024
D = 2048  # in features
O = 2048  # out features
E = 4
R = 8
ER = E * R  # 32
J = ER + E  # 36: rank-proj rows + router rows
DC = D // 128  # 16 contraction chunks
OCW = 512  # out-feature chunk width (one PSUM bank)
OC = O // OCW  # 4
TC = TPC // 128  # 8 token chunks per core
WAVE = 4  # wave-A token chunks (PSUM: WAVE + ytps1 + 2 filler groups + 1 ypsum)
KP = 128  # close-matmul contraction rows (33 used, padded to full array)

F32 = mybir.dt.float32
BF16 = mybir.dt.bfloat16

# Results of the last device run (for test harness inspection).
last_run_info: dict = {}

_cached = None


def _build_program():
    nc = bacc.Bacc()

    xt_d = nc.declare_dram_parameter("xt", [128, DC * TPC], BF16, isOutput=False)
    wt_d = nc.declare_dram_parameter("wt", [OC, 128, DC * OCW], BF16, isOutput=False)
    w1t_d = nc.declare_dram_parameter("w1t", [128, DC * J], BF16, isOutput=False)
    bcat_d = nc.declare_dram_parameter("bcat", [KP, O], BF16, isOutput=False)
    cst_d = nc.declare_dram_parameter("cst", [E, ER + 4], BF16, isOutput=False)
    out_d = nc.declare_dram_parameter("out", [OC, TC, 128, OCW], BF16, isOutput=True)

    MUL = mybir.AluOpType.mult

    with TileContext(nc) as tc:
        with (
            tc.tile_pool(name="cpool", bufs=1) as cpool,
            tc.tile_pool(name="wpool", bufs=4) as wpool,
            tc.tile_pool(name="opool", bufs=6) as opool,
            tc.tile_pool(name="mpsum", bufs=7, space="PSUM") as mpsum,
            tc.tile_pool(name="ypsum", bufs=1, space="PSUM") as ypsum,
        ):
            # x gets a dedicated ring (gpsimd) so its stream isn't halved by
            # the weight traffic; everything else rides the scalar ring in
            # consumption order (tables, w0 pieces, then bcat/w1..w3 which
            # are not needed before ~60us). Outputs go on sync.
            # The DMA subsystem ramps over the first ~15us, so the earliest
            # bytes must be exactly the critical-path ones. Sync ring: w1t
            # then the first x chunks (the Y-phase ladder), then x pairs;
            # outputs ride sync later. Scalar ring: w0 k-pieces (wave A),
            # then the x tail (disjoint tile - one tile written by two DMA
            # queues hangs the DGE), bcat/w1..w3, cst last.
            w1tr = cpool.tile([128, DC * J], BF16)
            nc.sync.dma_start(out=w1tr, in_=w1t_d[:, :])

            XSPL = 12
            xa = cpool.tile([128, XSPL * TPC], BF16)  # sync: dc0..11
            xb = cpool.tile([128, (DC - XSPL) * TPC], BF16)  # scalar: dc12-15

            def xsl(dc, a, b):
                if dc < XSPL:
                    return xa[:, dc * TPC + a : dc * TPC + b]
                return xb[:, (dc - XSPL) * TPC + a : (dc - XSPL) * TPC + b]

            def load_x(eng, out_ap, d0, nd):
                eng.dma_start(
                    out=out_ap, in_=xt_d[:, d0 * TPC : (d0 + nd) * TPC]
                )

            for dc in range(4):
                load_x(nc.sync, xa[:, dc * TPC : (dc + 1) * TPC], dc, 1)
            for dp in range(2, 6):
                load_x(
                    nc.sync, xa[:, 2 * dp * TPC : (2 * dp + 2) * TPC], 2 * dp, 2
                )

            wts = {
                oc: wpool.tile([128, DC * OCW], BF16, name=f"wtr{oc}", tag="wtr")
                for oc in range(OC)
            }
            for k in range(4):
                nc.scalar.dma_start(
                    out=wts[0][:, k * 4 * OCW : (k + 1) * 4 * OCW],
                    in_=wt_d[0, :, k * 4 * OCW : (k + 1) * 4 * OCW],
                )
            load_x(nc.scalar, xb[:, 0 : 2 * TPC], 12, 2)
            load_x(nc.scalar, xb[:, 2 * TPC : 4 * TPC], 14, 2)
            cstr = cpool.tile([E, ER + 4], BF16)
            nc.scalar.dma_start(out=cstr, in_=cst_d[:, :])
            bcatr = cpool.tile([KP, O], BF16)
            nc.scalar.dma_start(out=bcatr, in_=bcat_d[:, :])
            for oc in range(1, OC):
                for h in range(2):
                    nc.scalar.dma_start(
                        out=wts[oc][:, h * 8 * OCW : (h + 1) * 8 * OCW],
                        in_=wt_d[oc, :, h * 8 * OCW : (h + 1) * 8 * OCW],
                    )

            # Warmup: ramp the PE p-state on a zero tile while the first
            # x chunk is still in flight.
            warm_sb = cpool.tile([128, OCW], BF16)
            nc.vector.memset(warm_sb, 0.0)
            # Close lhsT padded to 128 contraction rows: 32 gated-proj rows,
            # the ones row (bias), then zeros (avoids 64-row PE tile mode).
            vwtr = cpool.tile([KP, TPC], BF16)
            for p0 in range(ER, KP, 32):
                nc.vector.memset(vwtr[p0 : p0 + 32, :], 0.0)
            nc.vector.memset(vwtr[ER : ER + 1, :], 1.0)
            warmps = ypsum.tile([128, OCW], F32, name="warmps", tag="yb")
            for _ in range(6):
                nc.tensor.matmul(
                    warmps, lhsT=warm_sb[:, 0:128], rhs=warm_sb, start=True, stop=True
                )

            # --- Router/LoRA-down phase: YT[j, tok] accumulated over all
            # dc, interleaved with wave-A base matmuls in DMA arrival order.
            # ytps[1] lives in the mpsum pool so ypsum (bufs=1) can rotate
            # the small gating-chain tiles behind it.
            ytps = [
                ypsum.tile([J, 512], F32, name="ytps0", tag="yb"),
                mpsum.tile([J, 512], F32, name="ytps1", tag="ps"),
            ]
            psA = {
                t: mpsum.tile([128, OCW], F32, name=f"ps0_{t}", tag="ps")
                for t in range(WAVE)
            }
            for k in range(4):
                for dc in range(4 * k, 4 * k + 4):
                    for th in range(2):
                        nc.tensor.matmul(
                            ytps[th],
                            lhsT=w1tr[:, dc * J : (dc + 1) * J],
                            rhs=xsl(dc, th * 512, (th + 1) * 512),
                            start=(dc == 0),
                            stop=(dc == DC - 1),
                        )
                for dc in range(4 * k, 4 * k + 4):
                    for t in range(WAVE):
                        nc.tensor.matmul(
                            psA[t],
                            lhsT=xsl(dc, t * 128, (t + 1) * 128),
                            rhs=wts[0][:, dc * OCW : (dc + 1) * OCW],
                            start=(dc == 0),
                            stop=False,
                        )

            # --- Gating chain (no PE transposes).
            # u = exp(logits) straight off PSUM (no max-sub: logits ~N(0,1));
            # sums4 = 0.5-ones matmul replicates s/SCALE onto 4 rows;
            # r4 = SCALE/s via fast approx recip; g = u*r4 on DVE;
            # gb = E8.T@g broadcasts per-expert gates to the 32 rank rows;
            # vw = Y * gb lands directly in the [32, tok] bf16 close layout.
            y_sb = cpool.tile([ER, TPC], BF16)
            u_sb = cpool.tile([E, TPC], BF16)
            r_sb = cpool.tile([E, TPC], F32)
            g_sb = cpool.tile([E, TPC], BF16)
            for th in range(2):
                nc.vector.tensor_copy(
                    y_sb[:, th * 512 : (th + 1) * 512], ytps[th][0:ER, :]
                )
                nc.scalar.activation(
                    u_sb[:, th * 512 : (th + 1) * 512],
                    ytps[th][ER:J, :],
                    mybir.ActivationFunctionType.Exp,
                )

            def open_group(wtr, t, name):
                ps = mpsum.tile([128, OCW], F32, name=name, tag="ps")
                for dc in range(DC):
                    nc.tensor.matmul(
                        ps,
                        lhsT=xsl(dc, t * 128, (t + 1) * 128),
                        rhs=wtr[:, dc * OCW : (dc + 1) * OCW],
                        start=(dc == 0),
                        stop=False,
                    )
                return ps

            def open_group_part(ps, wtr, t, dcs):
                for dc in dcs:
                    nc.tensor.matmul(
                        ps,
                        lhsT=xsl(dc, t * 128, (t + 1) * 128),
                        rhs=wtr[:, dc * OCW : (dc + 1) * OCW],
                        start=(dc == 0),
                        stop=False,
                    )

            # The t4/t5/t6 base groups need no new DMA data, so their 48
            # matmuls are interleaved as PE filler around the chain's small
            # matmuls; the single-buf ypsum rotation serializes the chain
            # (sums0 -> recip0 -> sums1 -> recip1 -> gb0 -> vw0 -> gb1 ->
            # vw1), each hop hidden behind ~1.7us of base matmuls.
            def sums_mm(th):
                s = ypsum.tile([E, 512], F32, name=f"sums{th}", tag="yb")
                nc.tensor.matmul(
                    s,
                    lhsT=cstr[0:E, ER : ER + 4],
                    rhs=u_sb[:, th * 512 : (th + 1) * 512],
                    start=True,
                    stop=True,
                )
                return s

            def recip_g(th, s):
                nc.vector.reciprocal_approx_fast(
                    out=r_sb[:, th * 512 : (th + 1) * 512], in_=s
                )
                nc.vector.tensor_tensor(
                    g_sb[:, th * 512 : (th + 1) * 512],
                    u_sb[:, th * 512 : (th + 1) * 512],
                    r_sb[:, th * 512 : (th + 1) * 512],
                    op=MUL,
                )

            def gb_mm(th):
                gb = ypsum.tile([ER, 512], F32, name=f"gb{th}", tag="yb")
                nc.tensor.matmul(
                    gb,
                    lhsT=cstr[0:E, 0:ER],
                    rhs=g_sb[:, th * 512 : (th + 1) * 512],
                    start=True,
                    stop=True,
                )
                return gb

            def vw_tt(th, gb):
                nc.vector.tensor_tensor(
                    vwtr[0:ER, th * 512 : (th + 1) * 512],
                    y_sb[:, th * 512 : (th + 1) * 512],
                    gb,
                    op=MUL,
                )

            psA[WAVE] = open_group(wts[0], WAVE, f"ps0_{WAVE}")
            s0 = sums_mm(0)
            recip_g(0, s0)
            psA[WAVE + 1] = mpsum.tile(
                [128, OCW], F32, name=f"ps0_{WAVE + 1}", tag="ps"
            )
            open_group_part(psA[WAVE + 1], wts[0], WAVE + 1, range(0, 8))
            s1 = sums_mm(1)
            recip_g(1, s1)
            open_group_part(psA[WAVE + 1], wts[0], WAVE + 1, range(8, DC))
            gb0 = gb_mm(0)
            vw_tt(0, gb0)
            psA[WAVE + 2] = mpsum.tile(
                [128, OCW], F32, name=f"ps0_{WAVE + 2}", tag="ps"
            )
            open_group_part(psA[WAVE + 2], wts[0], WAVE + 2, range(0, 8))
            gb1 = gb_mm(1)
            vw_tt(1, gb1)
            open_group_part(psA[WAVE + 2], wts[0], WAVE + 2, range(8, DC))

            # --- Close: fused LoRA-up + bias matmul, cast to bf16, DMA out.
            def close_group(ps, oc, t, last=False):
                nc.tensor.matmul(
                    ps,
                    lhsT=vwtr[:, t * 128 : (t + 1) * 128],
                    rhs=bcatr[:, oc * OCW : (oc + 1) * OCW],
                    start=False,
                    stop=True,
                )
                ot = opool.tile([128, OCW], BF16, tag="ot")
                if last:
                    # Pipeline the final tile's cast+DMA in halves (DVE and
                    # ACT in parallel) so the tail isn't one serial
                    # cast -> DMA chain; more splits lose to the ~0.6us
                    # per-DMA issue cost.
                    h = OCW // 2
                    nc.vector.tensor_copy(ot[:, 0:h], ps[:, 0:h])
                    nc.scalar.activation(
                        ot[:, h:OCW],
                        ps[:, h:OCW],
                        mybir.ActivationFunctionType.Copy,
                    )
                    nc.sync.dma_start(out=out_d[oc, t, :, 0:h], in_=ot[:, 0:h])
                    nc.sync.dma_start(
                        out=out_d[oc, t, :, h:OCW], in_=ot[:, h:OCW]
                    )
                else:
                    nc.vector.tensor_copy(ot, ps)
                    nc.sync.dma_start(out=out_d[oc, t], in_=ot)

            for t in range(WAVE + 3):
                close_group(psA[t], 0, t)
            for t in range(WAVE + 3, TC):
                close_group(open_group(wts[0], t, f"ps0_{t}"), 0, t)
            for oc in range(1, OC):
                for t in range(TC):
                    close_group(
                        open_group(wts[oc], t, f"ps{oc}_{t}"),
                        oc,
                        t,
                        last=(oc == OC - 1 and t == TC - 1),
                    )

    nc.compile()
    return nc


def _prep_inputs(x, base_w, base_b, A, B, router_w):
    """Host-side layout prep: per-partition-contiguous bf16 DMA images."""
    import ml_dtypes

    bf16 = ml_dtypes.bfloat16

    x2 = np.ascontiguousarray(x, dtype=np.float32).reshape(TOK, D)
    # xt[core][p, dc*TPC + t] = x2[core*TPC + t, dc*128 + p]
    xv = x2.reshape(N_CORES, TPC, DC, 128)
    xt = (
        np.ascontiguousarray(xv.transpose(0, 3, 2, 1))
        .reshape(N_CORES, 128, DC * TPC)
        .astype(bf16)
    )

    # wt[oc, p, dc*OCW + o] = base_w[oc*OCW + o, dc*128 + p]
    wv = np.ascontiguousarray(base_w, dtype=np.float32).reshape(OC, OCW, DC, 128)
    wt = (
        np.ascontiguousarray(wv.transpose(0, 3, 2, 1))
        .reshape(OC, 128, DC * OCW)
        .astype(bf16)
    )

    # W1 = [A flattened to 32 rows; router_w 4 rows] over D
    W1 = np.concatenate(
        [
            np.asarray(A, dtype=np.float32).reshape(ER, D),
            np.asarray(router_w, np.float32),
        ],
        axis=0,
    )  # [36, D]
    w1t = (
        np.ascontiguousarray(W1.reshape(J, DC, 128).transpose(2, 1, 0))
        .reshape(128, DC * J)
        .astype(bf16)
    )

    # bcat rows 0..31: B[e, o, r] -> [er, o]; row 32: base_b; rows 33..127
    # zero padding (close matmul runs with a full 128-row stationary tile).
    bc = np.zeros((KP, O), np.float32)
    bc[0:ER] = np.asarray(B, dtype=np.float32).transpose(0, 2, 1).reshape(ER, O)
    bc[ER] = np.asarray(base_b, dtype=np.float32)
    bc = bc.astype(bf16)  # [128, O]

    # cst[:, :32] = per-expert expansion (E8); cst[:, 32:36] = 1/SCALE ones
    # block so the sums matmul replicates s/SCALE onto 4 rows and the recip
    # gives SCALE/s directly.
    cst = np.zeros((E, ER + 4), np.float32)
    for e in range(E):
        cst[e, e * R : (e + 1) * R] = 1.0
    cst[:, ER : ER + 4] = 1.0 / SCALE
    cst = cst.astype(bf16)

    return xt, wt, w1t, bc, cst


def kernel(x, base_w, base_b, A, B, router_w):
    global _cached
    if _cached is None:
        _cached = _build_program()
    nc = _cached

    xt, wt, w1t, bc, cst = _prep_inputs(x, base_w, base_b, A, B, router_w)

    in_maps = [
        {"xt": xt[c], "wt": wt, "w1t": w1t, "bcat": bc, "cst": cst}
        for c in range(N_CORES)
    ]
    core_ids = list(range(N_CORES))

    profile = os.environ.get("KERNEL_PROFILE", "0") == "1"
    res = run_bass_kernel_spmd(nc, in_maps, core_ids, trace=profile)

    last_run_info.clear()
    last_run_info["exec_time_ns"] = res.exec_time_ns
    last_run_info["mean_exec_time_ns"] = res.mean_exec_time_ns
    last_run_info["instructions_and_trace"] = res.instructions_and_trace
    last_run_info["profile_json"] = res.profile_json

    # out[core] shape [OC, TC, 128, OCW] bf16 -> tokens x features fp32
    full = np.empty((TOK, O), dtype=np.float32)
    for c in range(N_CORES):
        buf = res.results[c]["out"].astype(np.float32)  # [OC, TC, 128, OCW]
        full[c * TPC : (c + 1) * TPC] = buf.transpose(1, 2, 0, 3).reshape(TPC, O)
    return full.reshape(4, 2048, 2048)
